# revision 17
# baseline (speedup 1.0000x reference)
"""Trainium2 Bass kernel for CrossAttention (nn_CrossAttention_82343112999000).

Reference computation (per batch b):
  q = x @ Wq.T; k = ctx @ Wk.T; v = ctx @ Wv.T     (nn.Linear, W stored [out, in])
  per head: attn = softmax(q k^T / sqrt(hd)); o = attn @ v
  out = concat_heads(o) @ Wo.T + bo + x

Sharding: pure data parallel over the 4096 flattened query rows.
Core c handles batch b = c//4 and query rows [(c%4)*512, (c%4+1)*512).
Each core computes the full k/v for its batch (duplicated across the 4 cores
of a batch; no collectives are needed).

Dataflow (per core), built around fp8e4 DoubleRow matmuls (2 contraction
tiles of 128 per instruction, 0.5 PE cycles per moving row):

  - x/Wq/Wk/Wv/ctx are cast fp32->fp8 into DRAM scratch, then transposed
    with the xbar DMA *viewed as uint16* (the xbar only does 2-byte moves):
    partition p of a transposed strip holds the adjacent fp8 column PAIR
    (2p, 2p+1), i.e. contraction index kappa = 256*tau + 2p + t.  A pair of
    strided engine copies de-interleaves each strip into the blocked
    [p, tau, t, n] layout DoubleRow needs.  Both operands of every
    projection matmul use the same kappa mapping, so the contraction is
    order-consistent.
  - projections: qT [e, m], kT [e, c] (bf16 copies out of fp32 PSUM for the
    scores), v natural [c, e] scattered per-head into vA8 (fp8) with an
    appended ones-column (attn@v then also emits the softmax denominator).
  - scores_h^T [c, m] = kT_h^T-tile @ qT_h in bf16 (hd=64 contraction; two
    heads of an e-tile use PE row groups 0-63 / 64-127).
  - P = exp(scale*scores - 2.5) on ACT, fp8 out ([128,1024] tiles); the bias
    keeps P inside fp8e4 range and cancels in the softmax normalization.
  - attn@v: 4 DoubleRow matmuls per head; DVE reciprocal + Pool
    partition-broadcast + DVE mul normalize into attnT8 (fp8, d-blocked).
  - out-projection: Wo goes through the bf16 xbar path and is cast on-chip
    to the d-blocked fp8 layout (matching attnT8's kappa); runs as
    DoubleRow in four d-quarters as heads complete, accumulating into the
    fp32 residual tile (x + bo); finished blocks stream out on ACT HWDGE.
"""

import numpy as np

import concourse.bass as bass
import concourse.tile as tile
from concourse import bacc, mybir
from concourse.bass_utils import run_bass_kernel_spmd

f32 = mybir.dt.float32
bf16 = mybir.dt.bfloat16
fp8 = mybir.dt.float8e4
u16 = mybir.dt.uint16
DR = mybir.MatmulPerfMode.DoubleRow
Exp = mybir.ActivationFunctionType.Exp

B, L, LC, D, CD, H, HD = 2, 2048, 1024, 1024, 768, 16, 64
NCORES = 8
M = (B * L) // NCORES  # 512 query rows per core
MT = M // 128  # 4
DT = D // 128  # 8
CDT = CD // 128  # 6
CT = LC // 128  # 8
ET = D // 128  # 8
TQ = D // 256  # 4  DoubleRow K-passes for d-contractions
TC = CD // 256  # 3  DoubleRow K-passes for cd-contractions
SCALE = float(HD) ** -0.5
VW = 80  # per-head v width in vA8: 64 v cols + ones col at 64 + 15 pad
LAG = 3  # attn@v for head h runs in slot h+LAG (hides exp latency)

LAST_RESULT = None  # BassKernelResults of the most recent run (for test.py)
_cached_nc = None


def _build():
    nc = bacc.Bacc("TRN2", target_bir_lowering=False, debug=False, num_devices=NCORES)
    x_d = nc.dram_tensor("x", [M, D], f32, kind="ExternalInput").ap()
    ctx_d = nc.dram_tensor("ctx", [LC, CD], f32, kind="ExternalInput").ap()
    wq_d = nc.dram_tensor("wq", [D, D], f32, kind="ExternalInput").ap()
    wk_d = nc.dram_tensor("wk", [D, CD], f32, kind="ExternalInput").ap()
    wv_d = nc.dram_tensor("wv", [D, CD], f32, kind="ExternalInput").ap()
    wo_d = nc.dram_tensor("wo", [D, D], f32, kind="ExternalInput").ap()
    bo_d = nc.dram_tensor("bo", [1, D], f32, kind="ExternalInput").ap()
    out_d = nc.dram_tensor("out", [M, D], f32, kind="ExternalOutput").ap()
    out_r = out_d.rearrange("(t p) d -> t p d", p=128)

    with tile.TileContext(nc) as tc:
        with (
            tc.tile_pool(name="const", bufs=1) as const_pool,
            tc.tile_pool(name="persist", bufs=1) as persist,
            tc.tile_pool(name="p8", bufs=LAG + 2) as p8_pool,
            tc.tile_pool(name="r", bufs=4) as r_pool,
            tc.tile_pool(name="dram", bufs=3, space="DRAM") as dram_pool,
            tc.tile_pool(name="mmps", bufs=2, space="PSUM") as mmps,
            tc.tile_pool(name="scps", bufs=2, space="PSUM") as scps,
            tc.tile_pool(name="avps", bufs=2, space="PSUM") as avps,
        ):
            # exp bias keeps P in fp8e4 range (max logit ~6.7 -> e^4.2 = 67
            # < 448); softmax normalization cancels it exactly
            ebias = const_pool.tile([128, 1], f32, tag="ebias")
            nc.gpsimd.memset(ebias[:], -2.5)

            bo_sb = const_pool.tile([1, D], f32, tag="bo")
            nc.scalar.dma_start(bo_sb[:], bo_d)
            bo_b = const_pool.tile([128, D], f32, tag="bo_b")
            nc.gpsimd.partition_broadcast(bo_b[:], bo_sb[:])

            # fp8 blocked operands: [p, tau, t, n] with kappa = 256*tau+2p+t
            x8 = persist.tile([128, TQ, 2, M], fp8, tag="x8")
            wq8 = persist.tile([128, TQ, 2, D], fp8, tag="wq8")
            wk8 = persist.tile([128, TC, 2, D], fp8, tag="wk8")
            wv8 = persist.tile([128, TC, 2, D], fp8, tag="wv8")
            ctx8 = persist.tile([128, TC, 2, LC], fp8, tag="ctx8")
            # bf16 attention operands (from PSUM)
            qT = persist.tile([128, ET, M], bf16, tag="qT")
            kT = persist.tile([128, ET, LC], bf16, tag="kT")
            vA8 = persist.tile([128, H, CT, VW], fp8, tag="vA8")
            attnT8 = persist.tile([128, 4, MT, 2, 128], fp8, tag="attnT8")
            # Wo: bf16 xbar path, then on-chip cast to d-blocked fp8
            woT = persist.tile([128, DT, D], bf16, tag="woT")
            wo8 = persist.tile([128, 4, 2, 2, 512], fp8, tag="wo8")
            acc = persist.tile([128, MT, D], f32, tag="acc")

            # vA8 pad columns zero, ones column at 64
            nc.gpsimd.memset(vA8[:, :, :, HD:], 0.0)
            nc.gpsimd.memset(vA8[:, :, :, HD : HD + 1], 1.0)

            # ---- fp8 scratch cast + u16 xbar transpose + de-interleave ----
            scratch = {}
            inter = {}

            def cast_cols(name, src_d, rows, width, lo, hi, after=None):
                if name not in scratch:
                    scratch[name] = dram_pool.tile(
                        [rows, width], fp8, tag="scr", name=f"scr_{name}"
                    )
                c = nc.gpsimd.dma_start(
                    scratch[name][:, lo:hi], src_d[:, lo:hi]
                )
                if after is not None:
                    tile.add_dep_helper(c.ins, after, reason="dma priority")
                return c.ins

            def transpose_deint(name, dest, rows, width, slo, shi, eng):
                """u16 transpose of strips [256*slo, 256*shi), then one
                strided copy per strip de-interleaves into dest[:,tau,:,:]."""
                scr_u = scratch[name][:].bitcast(u16)  # [rows, width//2]
                nt = width // 256
                if name not in inter:
                    inter[name] = persist.tile(
                        [128, nt, 2 * rows], fp8, tag="inter", name=f"i8_{name}"
                    )
                it = inter[name]
                tr = nc.sync.dma_start_transpose(
                    out=it[:, slo:shi, :].bitcast(u16),
                    in_=scr_u[:, slo * 128 : shi * 128],
                )
                for tau in range(slo, shi):
                    iv = it[:, tau, :].rearrange("p (n t) -> p t n", t=2)
                    if eng is nc.scalar:
                        nc.scalar.copy(dest[:, tau, :, :], iv[:, :, :])
                    else:
                        eng.tensor_copy(dest[:, tau, :, :], iv[:, :, :])
                return tr.ins

            # DMA priority chain: q path (x, wq) first so PE starts early;
            # later casts are dep-held so their transfers cannot jump the
            # single DMA device's queue ahead of critical transposes.
            cast_cols("x", x_d, M, D, 0, 512)
            cast_cols("wq", wq_d, D, D, 0, 512)
            t_x0 = transpose_deint("x", x8, M, D, 0, 2, nc.scalar)
            t_wq0 = transpose_deint("wq", wq8, D, D, 0, 2, nc.scalar)
            cast_cols("x", x_d, M, D, 512, D, after=t_x0)
            cast_cols("wq", wq_d, D, D, 512, D, after=t_wq0)
            t_x1 = transpose_deint("x", x8, M, D, 2, 4, nc.scalar)
            t_wq1 = transpose_deint("wq", wq8, D, D, 2, 4, nc.scalar)
            cast_cols("wk", wk_d, D, CD, 0, CD, after=t_x1)
            cast_cols("ctx", ctx_d, LC, CD, 0, CD, after=t_wq1)
            t_wk = transpose_deint("wk", wk8, D, CD, 0, 3, nc.gpsimd)
            t_ctx = transpose_deint("ctx", ctx8, LC, CD, 0, 3, nc.gpsimd)
            cast_cols("wv", wv_d, D, CD, 0, CD, after=t_wk)
            t_wv = transpose_deint("wv", wv8, D, CD, 0, 3, nc.vector)

            # Wo bf16 path (cast -> xbar -> SBUF) and the residual load —
            # needed only mid-attention, held behind the wv transpose
            wo_scr = dram_pool.tile([D, D], bf16, tag="wo_scr")
            c_wo = nc.gpsimd.dma_start(wo_scr[:], wo_d)
            tile.add_dep_helper(c_wo.ins, t_wv, reason="dma priority")
            nc.sync.dma_start_transpose(out=woT[:], in_=wo_scr[:])
            c_acc = nc.scalar.dma_start(
                acc[:], x_d.rearrange("(t p) d -> p t d", p=128)
            )
            tile.add_dep_helper(c_acc.ins, t_wv, reason="dma priority")
            for mt in range(MT):
                nc.gpsimd.tensor_add(acc[:, mt, :], acc[:, mt, :], bo_b[:])

            # ---- fp8 DoubleRow projection chains ----
            def emit_q(et):
                ps = mmps.tile([128, M], f32, tag="ps", name="qps")
                for tau in range(TQ):
                    nc.tensor.matmul(
                        ps[:],
                        wq8[:, tau, :, et * 128 : (et + 1) * 128],
                        x8[:, tau, :, :],
                        start=(tau == 0),
                        stop=(tau == TQ - 1),
                        perf_mode=DR,
                    )
                if et < 3:
                    nc.scalar.copy(qT[:, et, :], ps[:])
                else:
                    nc.vector.tensor_copy(qT[:, et, :], ps[:])

            def emit_k(et, cc):
                ps = mmps.tile([128, 512], f32, tag="ps", name="kps")
                for tau in range(TC):
                    nc.tensor.matmul(
                        ps[:],
                        wk8[:, tau, :, et * 128 : (et + 1) * 128],
                        ctx8[:, tau, :, cc * 512 : (cc + 1) * 512],
                        start=(tau == 0),
                        stop=(tau == TC - 1),
                        perf_mode=DR,
                    )
                if et < 1:
                    nc.scalar.copy(kT[:, et, cc * 512 : (cc + 1) * 512], ps[:])
                else:
                    nc.vector.tensor_copy(kT[:, et, cc * 512 : (cc + 1) * 512], ps[:])

            def emit_v(ct, ec):
                ps = mmps.tile([128, 512], f32, tag="ps", name="vps")
                for tau in range(TC):
                    nc.tensor.matmul(
                        ps[:],
                        ctx8[:, tau, :, ct * 128 : (ct + 1) * 128],
                        wv8[:, tau, :, ec * 512 : (ec + 1) * 512],
                        start=(tau == 0),
                        stop=(tau == TC - 1),
                        perf_mode=DR,
                    )
                nc.vector.tensor_copy(
                    vA8[:, ec * 8 : (ec + 1) * 8, ct, 0:HD],
                    ps[:].rearrange("p (h w) -> p h w", w=HD),
                )

            def emit_wo8(tau, ec):
                nc.gpsimd.tensor_copy(
                    wo8[:, tau, ec, :, :],
                    woT[:, 2 * tau : 2 * tau + 2, ec * 512 : (ec + 1) * 512],
                )

            # ---- attention emitters ----
            p8_tiles = {}

            def emit_scores_exp(h):
                et, half = h // 2, h % 2
                rows = slice(half * HD, (half + 1) * HD)
                p8 = p8_pool.tile([128, TQ, LC], fp8, tag="p8", name="p8t")
                p8_tiles[h] = p8
                for ctp in range(4):
                    sc = scps.tile([128, 1024], f32, tag="sc", name="scs")
                    for k2 in range(2):
                        ct = 2 * ctp + k2
                        nc.tensor.matmul(
                            sc[:, k2 * 512 : (k2 + 1) * 512],
                            kT[rows, et, ct * 128 : (ct + 1) * 128],
                            qT[rows, et, :],
                            start=True,
                            stop=True,
                        )
                    nc.scalar.activation(
                        out=p8[:, ctp, :], in_=sc[:], func=Exp, scale=SCALE,
                        bias=ebias[:],
                    )

            def emit_av_norm(h):
                rows = slice((h % 2) * HD, (h % 2 + 1) * HD)
                p8 = p8_tiles.pop(h)
                av = avps.tile([VW, 512], f32, tag="av", name="avt")
                for tau in range(4):
                    nc.tensor.matmul(
                        av[:],
                        vA8[:, h, 2 * tau : 2 * tau + 2, :],
                        p8[:, tau, :].rearrange("p (t m) -> p t m", t=2),
                        start=(tau == 0),
                        stop=(tau == 3),
                        perf_mode=DR,
                    )
                rcp = r_pool.tile([1, 512], f32, tag="r")
                nc.vector.reciprocal(rcp[:], av[HD : HD + 1, :])
                rcp_b = r_pool.tile([HD, 512], f32, tag="rb")
                nc.gpsimd.partition_broadcast(rcp_b[:], rcp[:])
                tau, t = h // 4, (h % 4) // 2
                nc.vector.tensor_mul(
                    attnT8[rows, tau, :, t, :],
                    av[0:HD, :].rearrange("p (mt j) -> p mt j", j=128),
                    rcp_b[:].rearrange("p (mt j) -> p mt j", j=128),
                )

            def out_piece(tau, piece):
                mt, ec = piece // 2, piece % 2
                ps = mmps.tile([128, 512], f32, tag="ps", name="ops")
                nc.tensor.matmul(
                    ps[:],
                    attnT8[:, tau, mt, :, :],
                    wo8[:, tau, ec, :, :],
                    start=True,
                    stop=True,
                    perf_mode=DR,
                )
                sl = slice(ec * 512, (ec + 1) * 512)
                nc.vector.tensor_add(acc[:, mt, sl], acc[:, mt, sl], ps[:])
                if tau == 3:
                    nc.scalar.dma_start(out_r[mt][:, sl], acc[:, mt, sl])

            # ---- PE program ----
            # preamble: q chains (strip deps let tau-passes start as x/wq
            # strips land), k et0-1, v ec0 ct0-3
            for et in range(ET):
                emit_q(et)
            for et in range(2):
                emit_k(et, 0)
                emit_k(et, 1)
            for ct in range(4):
                emit_v(ct, 0)

            chains = {
                0: [lambda: emit_v(4, 0), lambda: emit_v(5, 0)],
                1: [lambda: emit_v(6, 0), lambda: emit_v(7, 0)],
                2: [lambda: emit_k(2, 0), lambda: emit_k(2, 1)],
                3: [lambda: emit_k(3, 0), lambda: emit_k(3, 1)],
                4: [lambda: emit_v(0, 1), lambda: emit_v(1, 1)],
                5: [lambda: emit_v(2, 1), lambda: emit_v(3, 1),
                    lambda: emit_wo8(0, 0), lambda: emit_wo8(0, 1)],
                6: [lambda: emit_k(4, 0), lambda: emit_k(4, 1),
                    lambda: emit_wo8(1, 0), lambda: emit_wo8(1, 1)],
                7: [lambda: emit_v(4, 1), lambda: emit_v(5, 1),
                    lambda: emit_wo8(2, 0), lambda: emit_wo8(2, 1)],
                8: [lambda: emit_v(6, 1), lambda: emit_v(7, 1),
                    lambda: emit_wo8(3, 0), lambda: emit_wo8(3, 1)],
                9: [lambda: emit_k(5, 0), lambda: emit_k(5, 1)],
                10: [lambda: emit_k(6, 0), lambda: emit_k(6, 1)],
                11: [lambda: emit_k(7, 0), lambda: emit_k(7, 1)],
            }
            pieces = {
                7: [(0, 0), (0, 1), (0, 2), (0, 3)],
                8: [(0, 4), (0, 5), (0, 6), (0, 7)],
                11: [(1, 0), (1, 1), (1, 2), (1, 3)],
                12: [(1, 4), (1, 5), (1, 6), (1, 7)],
                15: [(2, 0), (2, 1), (2, 2), (2, 3)],
                16: [(2, 4), (2, 5), (2, 6), (2, 7)],
                18: [(3, 0), (3, 1), (3, 2), (3, 3)],
                19: [(3, 4), (3, 5), (3, 6), (3, 7)],
            }

            for slot in range(H + LAG + 2):
                if slot < H:
                    emit_scores_exp(slot)
                for fn in chains.get(slot, ()):
                    fn()
                if slot >= LAG and slot - LAG < H:
                    emit_av_norm(slot - LAG)
                for tau, piece in pieces.get(slot, ()):
                    out_piece(tau, piece)

    nc.compile()
    return nc


def kernel(x, context, Wq, Wk, Wv, Wo, bo):
    global LAST_RESULT, _cached_nc
    if _cached_nc is None:
        _cached_nc = _build()
    nc = _cached_nc

    x = np.ascontiguousarray(x, dtype=np.float32)
    context = np.ascontiguousarray(context, dtype=np.float32)
    wq = np.ascontiguousarray(Wq, dtype=np.float32)
    wk = np.ascontiguousarray(Wk, dtype=np.float32)
    wv = np.ascontiguousarray(Wv, dtype=np.float32)
    wo = np.ascontiguousarray(Wo, dtype=np.float32)
    bo2 = np.ascontiguousarray(bo, dtype=np.float32).reshape(1, D)

    in_maps = []
    for c in range(NCORES):
        b = c // (NCORES // B)
        ls = (c % (NCORES // B)) * M
        in_maps.append(
            {
                "x": np.ascontiguousarray(x[b, ls : ls + M, :]),
                "ctx": context[b],
                "wq": wq,
                "wk": wk,
                "wv": wv,
                "wo": wo,
                "bo": bo2,
            }
        )

    res = run_bass_kernel_spmd(nc, in_maps, core_ids=list(range(NCORES)))
    LAST_RESULT = res

    out = np.empty((B, L, D), dtype=np.float32)
    for c in range(NCORES):
        b = c // (NCORES // B)
        ls = (c % (NCORES // B)) * M
        out[b, ls : ls + M, :] = res.results[c]["out"]
    return out


# revision 18
# speedup vs baseline: 1.1606x; 1.1606x over previous
"""Trainium2 Bass kernel for CrossAttention (nn_CrossAttention_82343112999000).

Reference computation (per batch b):
  q = x @ Wq.T; k = ctx @ Wk.T; v = ctx @ Wv.T     (nn.Linear, W stored [out, in])
  per head: attn = softmax(q k^T / sqrt(hd)); o = attn @ v
  out = concat_heads(o) @ Wo.T + bo + x

Sharding: pure data parallel over the 4096 flattened query rows.
Core c handles batch b = c//4 and query rows [(c%4)*512, (c%4+1)*512).
Each core computes the full k/v for its batch (duplicated across the 4 cores
of a batch; no collectives are needed).

Dataflow (per core), built around fp8e4 DoubleRow matmuls (2 contraction
tiles of 128 per instruction, 0.5 PE cycles per moving row):

  - x/Wq/Wk/Wv/ctx are cast fp32->fp8 into DRAM scratch, then transposed
    with the xbar DMA *viewed as uint16* (the xbar only does 2-byte moves):
    partition p of a transposed strip holds the adjacent fp8 column PAIR
    (2p, 2p+1), i.e. contraction index kappa = 256*tau + 2p + t.  A pair of
    strided engine copies de-interleaves each strip into the blocked
    [p, tau, t, n] layout DoubleRow needs.  Both operands of every
    projection matmul use the same kappa mapping, so the contraction is
    order-consistent.
  - projections: qT [e, m], kT [e, c] (bf16 copies out of fp32 PSUM for the
    scores), v natural [c, e] scattered per-head into vA8 (fp8) with an
    appended ones-column (attn@v then also emits the softmax denominator).
  - scores_h^T [c, m] = kT_h^T-tile @ qT_h in bf16 (hd=64 contraction; two
    heads of an e-tile use PE row groups 0-63 / 64-127).
  - P = exp(scale*scores - 2.5) on ACT, fp8 out ([128,1024] tiles); the bias
    keeps P inside fp8e4 range and cancels in the softmax normalization.
  - attn@v: 4 DoubleRow matmuls per head; DVE reciprocal + Pool
    partition-broadcast + DVE mul normalize into attnT8 (fp8, d-blocked).
  - out-projection: Wo goes through the bf16 xbar path and is cast on-chip
    to the d-blocked fp8 layout (matching attnT8's kappa); runs as
    DoubleRow in four d-quarters as heads complete, accumulating into the
    fp32 residual tile (x + bo); finished blocks stream out on ACT HWDGE.
"""

import numpy as np

import concourse.bass as bass
import concourse.tile as tile
from concourse import bacc, mybir
from concourse.bass_utils import run_bass_kernel_spmd

f32 = mybir.dt.float32
bf16 = mybir.dt.bfloat16
fp8 = mybir.dt.float8e4
u16 = mybir.dt.uint16
DR = mybir.MatmulPerfMode.DoubleRow
Exp = mybir.ActivationFunctionType.Exp

B, L, LC, D, CD, H, HD = 2, 2048, 1024, 1024, 768, 16, 64
NCORES = 8
M = (B * L) // NCORES  # 512 query rows per core
MT = M // 128  # 4
DT = D // 128  # 8
CDT = CD // 128  # 6
CT = LC // 128  # 8
ET = D // 128  # 8
TQ = D // 256  # 4  DoubleRow K-passes for d-contractions
TC = CD // 256  # 3  DoubleRow K-passes for cd-contractions
SCALE = float(HD) ** -0.5
VW = 80  # per-head v width in vA8: 64 v cols + ones col at 64 + 15 pad
LAG = 3  # attn@v for head h runs in slot h+LAG (hides exp latency)

LAST_RESULT = None  # BassKernelResults of the most recent run (for test.py)
_cached_nc = None


def _build():
    nc = bacc.Bacc("TRN2", target_bir_lowering=False, debug=False, num_devices=NCORES)
    x_d = nc.dram_tensor("x", [M, D], f32, kind="ExternalInput").ap()
    ctx_d = nc.dram_tensor("ctx", [LC, CD], f32, kind="ExternalInput").ap()
    wq_d = nc.dram_tensor("wq", [D, D], f32, kind="ExternalInput").ap()
    wk_d = nc.dram_tensor("wk", [D, CD], f32, kind="ExternalInput").ap()
    wv_d = nc.dram_tensor("wv", [D, CD], f32, kind="ExternalInput").ap()
    wo_d = nc.dram_tensor("wo", [D, D], f32, kind="ExternalInput").ap()
    bo_d = nc.dram_tensor("bo", [1, D], f32, kind="ExternalInput").ap()
    out_d = nc.dram_tensor("out", [M, D], f32, kind="ExternalOutput").ap()
    out_r = out_d.rearrange("(t p) d -> t p d", p=128)

    with tile.TileContext(nc) as tc:
        with (
            tc.tile_pool(name="const", bufs=1) as const_pool,
            tc.tile_pool(name="persist", bufs=1) as persist,
            tc.tile_pool(name="p8", bufs=LAG + 2) as p8_pool,
            tc.tile_pool(name="r", bufs=4) as r_pool,
            tc.tile_pool(name="dram", bufs=3, space="DRAM") as dram_pool,
            tc.tile_pool(name="mmps", bufs=2, space="PSUM") as mmps,
            tc.tile_pool(name="scps", bufs=2, space="PSUM") as scps,
            tc.tile_pool(name="avps", bufs=2, space="PSUM") as avps,
        ):
            # exp bias keeps P in fp8e4 range (max logit ~6.7 -> e^4.2 = 67
            # < 448); softmax normalization cancels it exactly
            ebias = const_pool.tile([128, 1], f32, tag="ebias")
            nc.gpsimd.memset(ebias[:], -2.5)

            bo_sb = const_pool.tile([1, D], f32, tag="bo")
            nc.scalar.dma_start(bo_sb[:], bo_d)
            bo_b = const_pool.tile([128, D], f32, tag="bo_b")

            # fp8 blocked operands: [p, tau, t, n] with kappa = 256*tau+2p+t
            x8 = persist.tile([128, TQ, 2, M], fp8, tag="x8")
            wq8 = persist.tile([128, TQ, 2, D], fp8, tag="wq8")
            wk8 = persist.tile([128, TC, 2, D], fp8, tag="wk8")
            wv8 = persist.tile([128, TC, 2, D], fp8, tag="wv8")
            ctx8 = persist.tile([128, TC, 2, LC], fp8, tag="ctx8")
            # bf16 attention operands (from PSUM)
            qT = persist.tile([128, ET, M], bf16, tag="qT")
            kT = persist.tile([128, ET, LC], bf16, tag="kT")
            vA8 = persist.tile([128, H, CT, VW], fp8, tag="vA8")
            attnT8 = persist.tile([128, 4, MT, 2, 128], fp8, tag="attnT8")
            # Wo: bf16 xbar path, then on-chip cast to d-blocked fp8
            woT = persist.tile([128, DT, D], bf16, tag="woT")
            wo8 = persist.tile([128, 4, 2, 2, 512], fp8, tag="wo8")
            acc = persist.tile([128, MT, D], f32, tag="acc")

            # ---- fp8 scratch cast + u16 xbar transpose + de-interleave ----
            scratch = {}
            inter = {}

            def cast_cols(name, src_d, rows, width, lo, hi, after=None):
                if name not in scratch:
                    scratch[name] = dram_pool.tile(
                        [rows, width], fp8, tag="scr", name=f"scr_{name}"
                    )
                c = nc.gpsimd.dma_start(
                    scratch[name][:, lo:hi], src_d[:, lo:hi]
                )
                if after is not None:
                    tile.add_dep_helper(c.ins, after, reason="dma priority")
                return c.ins

            def transpose_deint(name, dest, rows, width, slo, shi, eng):
                """u16 transpose of strips [256*slo, 256*shi), then one
                strided copy per strip de-interleaves into dest[:,tau,:,:]."""
                scr_u = scratch[name][:].bitcast(u16)  # [rows, width//2]
                nt = width // 256
                if name not in inter:
                    inter[name] = persist.tile(
                        [128, nt, 2 * rows], fp8, tag="inter", name=f"i8_{name}"
                    )
                it = inter[name]
                tr = nc.sync.dma_start_transpose(
                    out=it[:, slo:shi, :].bitcast(u16),
                    in_=scr_u[:, slo * 128 : shi * 128],
                )
                for tau in range(slo, shi):
                    iv = it[:, tau, :].rearrange("p (n t) -> p t n", t=2)
                    if eng is nc.scalar:
                        nc.scalar.copy(dest[:, tau, :, :], iv[:, :, :])
                    else:
                        eng.tensor_copy(dest[:, tau, :, :], iv[:, :, :])
                return tr.ins

            # DMA priority chain: q path (x, wq) first so PE starts early;
            # later casts are dep-held so their transfers cannot jump the
            # single DMA device's queue ahead of critical transposes.
            cast_cols("x", x_d, M, D, 0, 512)
            cast_cols("wq", wq_d, D, D, 0, 512)
            t_x0 = transpose_deint("x", x8, M, D, 0, 2, nc.scalar)
            t_wq0 = transpose_deint("wq", wq8, D, D, 0, 2, nc.scalar)
            cast_cols("x", x_d, M, D, 512, D)
            cast_cols("wq", wq_d, D, D, 512, D)
            t_x1 = transpose_deint("x", x8, M, D, 2, 4, nc.scalar)
            t_wq1 = transpose_deint("wq", wq8, D, D, 2, 4, nc.scalar)
            cast_cols("wk", wk_d, D, CD, 0, CD, after=t_x1)
            cast_cols("ctx", ctx_d, LC, CD, 0, CD, after=t_wq1)
            t_wk = transpose_deint("wk", wk8, D, CD, 0, 3, nc.vector)
            t_ctx = transpose_deint("ctx", ctx8, LC, CD, 0, 3, nc.vector)
            cast_cols("wv", wv_d, D, CD, 0, CD, after=t_wk)
            t_wv = transpose_deint("wv", wv8, D, CD, 0, 3, nc.vector)

            # Wo bf16 path (cast -> xbar -> SBUF) and the residual load —
            # needed only mid-attention, held behind the wv transpose
            wo_scr = dram_pool.tile([D, D], bf16, tag="wo_scr")
            c_wo = nc.gpsimd.dma_start(wo_scr[:], wo_d)
            tile.add_dep_helper(c_wo.ins, t_wv, reason="dma priority")
            nc.sync.dma_start_transpose(out=woT[:], in_=wo_scr[:])
            c_acc = nc.scalar.dma_start(
                acc[:], x_d.rearrange("(t p) d -> p t d", p=128)
            )
            tile.add_dep_helper(c_acc.ins, t_wv, reason="dma priority")
            nc.gpsimd.partition_broadcast(bo_b[:], bo_sb[:])
            # vA8 pad columns zero, ones column at 64
            nc.gpsimd.memset(vA8[:, :, :, HD:], 0.0)
            nc.gpsimd.memset(vA8[:, :, :, HD : HD + 1], 1.0)
            for mt in range(MT):
                nc.gpsimd.tensor_add(acc[:, mt, :], acc[:, mt, :], bo_b[:])

            # ---- fp8 DoubleRow projection chains ----
            def emit_q(et):
                ps = mmps.tile([128, M], f32, tag="ps", name="qps")
                for tau in range(TQ):
                    nc.tensor.matmul(
                        ps[:],
                        wq8[:, tau, :, et * 128 : (et + 1) * 128],
                        x8[:, tau, :, :],
                        start=(tau == 0),
                        stop=(tau == TQ - 1),
                        perf_mode=DR,
                    )
                if et < 3:
                    nc.scalar.copy(qT[:, et, :], ps[:])
                else:
                    nc.vector.tensor_copy(qT[:, et, :], ps[:])

            def emit_k(et, cc):
                ps = mmps.tile([128, 512], f32, tag="ps", name="kps")
                for tau in range(TC):
                    nc.tensor.matmul(
                        ps[:],
                        wk8[:, tau, :, et * 128 : (et + 1) * 128],
                        ctx8[:, tau, :, cc * 512 : (cc + 1) * 512],
                        start=(tau == 0),
                        stop=(tau == TC - 1),
                        perf_mode=DR,
                    )
                if et < 1:
                    nc.scalar.copy(kT[:, et, cc * 512 : (cc + 1) * 512], ps[:])
                else:
                    nc.vector.tensor_copy(kT[:, et, cc * 512 : (cc + 1) * 512], ps[:])

            def emit_v(ct, ec):
                ps = mmps.tile([128, 512], f32, tag="ps", name="vps")
                for tau in range(TC):
                    nc.tensor.matmul(
                        ps[:],
                        ctx8[:, tau, :, ct * 128 : (ct + 1) * 128],
                        wv8[:, tau, :, ec * 512 : (ec + 1) * 512],
                        start=(tau == 0),
                        stop=(tau == TC - 1),
                        perf_mode=DR,
                    )
                nc.vector.tensor_copy(
                    vA8[:, ec * 8 : (ec + 1) * 8, ct, 0:HD],
                    ps[:].rearrange("p (h w) -> p h w", w=HD),
                )

            def emit_wo8(tau, ec):
                nc.gpsimd.tensor_copy(
                    wo8[:, tau, ec, :, :],
                    woT[:, 2 * tau : 2 * tau + 2, ec * 512 : (ec + 1) * 512],
                )

            # ---- attention emitters ----
            p8_tiles = {}

            def emit_scores_exp(h):
                et, half = h // 2, h % 2
                rows = slice(half * HD, (half + 1) * HD)
                p8 = p8_pool.tile([128, TQ, LC], fp8, tag="p8", name="p8t")
                p8_tiles[h] = p8
                for ctp in range(4):
                    sc = scps.tile([128, 1024], f32, tag="sc", name="scs")
                    for k2 in range(2):
                        ct = 2 * ctp + k2
                        nc.tensor.matmul(
                            sc[:, k2 * 512 : (k2 + 1) * 512],
                            kT[rows, et, ct * 128 : (ct + 1) * 128],
                            qT[rows, et, :],
                            start=True,
                            stop=True,
                        )
                    nc.scalar.activation(
                        out=p8[:, ctp, :], in_=sc[:], func=Exp, scale=SCALE,
                        bias=ebias[:],
                    )

            def emit_av_norm(h):
                rows = slice((h % 2) * HD, (h % 2 + 1) * HD)
                p8 = p8_tiles.pop(h)
                av = avps.tile([VW, 512], f32, tag="av", name="avt")
                for tau in range(4):
                    nc.tensor.matmul(
                        av[:],
                        vA8[:, h, 2 * tau : 2 * tau + 2, :],
                        p8[:, tau, :].rearrange("p (t m) -> p t m", t=2),
                        start=(tau == 0),
                        stop=(tau == 3),
                        perf_mode=DR,
                    )
                rcp = r_pool.tile([1, 512], f32, tag="r")
                nc.vector.reciprocal(rcp[:], av[HD : HD + 1, :])
                rcp_b = r_pool.tile([HD, 512], f32, tag="rb")
                nc.gpsimd.partition_broadcast(rcp_b[:], rcp[:])
                tau, t = h // 4, (h % 4) // 2
                nc.vector.tensor_mul(
                    attnT8[rows, tau, :, t, :],
                    av[0:HD, :].rearrange("p (mt j) -> p mt j", j=128),
                    rcp_b[:].rearrange("p (mt j) -> p mt j", j=128),
                )

            def out_piece(tau, piece):
                mt, ec = piece // 2, piece % 2
                ps = mmps.tile([128, 512], f32, tag="ps", name="ops")
                nc.tensor.matmul(
                    ps[:],
                    attnT8[:, tau, mt, :, :],
                    wo8[:, tau, ec, :, :],
                    start=True,
                    stop=True,
                    perf_mode=DR,
                )
                sl = slice(ec * 512, (ec + 1) * 512)
                nc.vector.tensor_add(acc[:, mt, sl], acc[:, mt, sl], ps[:])
                if tau == 3:
                    nc.scalar.dma_start(out_r[mt][:, sl], acc[:, mt, sl])

            # ---- PE program ----
            # preamble: q chains (strip deps let tau-passes start as x/wq
            # strips land), k et0-1, v ec0 ct0-3
            for et in range(ET):
                emit_q(et)
            for et in range(2):
                emit_k(et, 0)
                emit_k(et, 1)
            for ct in range(4):
                emit_v(ct, 0)

            chains = {
                0: [lambda: emit_v(4, 0), lambda: emit_v(5, 0)],
                1: [lambda: emit_v(6, 0), lambda: emit_v(7, 0)],
                2: [lambda: emit_k(2, 0), lambda: emit_k(2, 1)],
                3: [lambda: emit_k(3, 0), lambda: emit_k(3, 1)],
                4: [lambda: emit_v(0, 1), lambda: emit_v(1, 1)],
                5: [lambda: emit_v(2, 1), lambda: emit_v(3, 1),
                    lambda: emit_wo8(0, 0), lambda: emit_wo8(0, 1)],
                6: [lambda: emit_k(4, 0), lambda: emit_k(4, 1),
                    lambda: emit_wo8(1, 0), lambda: emit_wo8(1, 1)],
                7: [lambda: emit_v(4, 1), lambda: emit_v(5, 1),
                    lambda: emit_wo8(2, 0), lambda: emit_wo8(2, 1)],
                8: [lambda: emit_v(6, 1), lambda: emit_v(7, 1),
                    lambda: emit_wo8(3, 0), lambda: emit_wo8(3, 1)],
                9: [lambda: emit_k(5, 0), lambda: emit_k(5, 1)],
                10: [lambda: emit_k(6, 0), lambda: emit_k(6, 1)],
                11: [lambda: emit_k(7, 0), lambda: emit_k(7, 1)],
            }
            pieces = {
                7: [(0, 0), (0, 1), (0, 2), (0, 3)],
                8: [(0, 4), (0, 5), (0, 6), (0, 7)],
                11: [(1, 0), (1, 1), (1, 2), (1, 3)],
                12: [(1, 4), (1, 5), (1, 6), (1, 7)],
                15: [(2, 0), (2, 1), (2, 2), (2, 3)],
                16: [(2, 4), (2, 5), (2, 6), (2, 7)],
                18: [(3, 0), (3, 1), (3, 2), (3, 3)],
                19: [(3, 4), (3, 5), (3, 6), (3, 7)],
            }

            for slot in range(H + LAG + 2):
                if slot < H:
                    emit_scores_exp(slot)
                for fn in chains.get(slot, ()):
                    fn()
                if slot >= LAG and slot - LAG < H:
                    emit_av_norm(slot - LAG)
                for tau, piece in pieces.get(slot, ()):
                    out_piece(tau, piece)

    nc.compile()
    return nc


def kernel(x, context, Wq, Wk, Wv, Wo, bo):
    global LAST_RESULT, _cached_nc
    if _cached_nc is None:
        _cached_nc = _build()
    nc = _cached_nc

    x = np.ascontiguousarray(x, dtype=np.float32)
    context = np.ascontiguousarray(context, dtype=np.float32)
    wq = np.ascontiguousarray(Wq, dtype=np.float32)
    wk = np.ascontiguousarray(Wk, dtype=np.float32)
    wv = np.ascontiguousarray(Wv, dtype=np.float32)
    wo = np.ascontiguousarray(Wo, dtype=np.float32)
    bo2 = np.ascontiguousarray(bo, dtype=np.float32).reshape(1, D)

    in_maps = []
    for c in range(NCORES):
        b = c // (NCORES // B)
        ls = (c % (NCORES // B)) * M
        in_maps.append(
            {
                "x": np.ascontiguousarray(x[b, ls : ls + M, :]),
                "ctx": context[b],
                "wq": wq,
                "wk": wk,
                "wv": wv,
                "wo": wo,
                "bo": bo2,
            }
        )

    res = run_bass_kernel_spmd(nc, in_maps, core_ids=list(range(NCORES)))
    LAST_RESULT = res

    out = np.empty((B, L, D), dtype=np.float32)
    for c in range(NCORES):
        b = c // (NCORES // B)
        ls = (c % (NCORES // B)) * M
        out[b, ls : ls + M, :] = res.results[c]["out"]
    return out


# revision 21
# speedup vs baseline: 1.1697x; 1.0078x over previous
"""Trainium2 Bass kernel for CrossAttention (nn_CrossAttention_82343112999000).

Reference computation (per batch b):
  q = x @ Wq.T; k = ctx @ Wk.T; v = ctx @ Wv.T     (nn.Linear, W stored [out, in])
  per head: attn = softmax(q k^T / sqrt(hd)); o = attn @ v
  out = concat_heads(o) @ Wo.T + bo + x

Sharding: pure data parallel over the 4096 flattened query rows.
Core c handles batch b = c//4 and query rows [(c%4)*512, (c%4+1)*512).
Each core computes the full k/v for its batch (duplicated across the 4 cores
of a batch; no collectives are needed).

Dataflow (per core), built around fp8e4 DoubleRow matmuls (2 contraction
tiles of 128 per instruction, 0.5 PE cycles per moving row):

  - x/Wq/Wk/Wv/ctx are cast fp32->fp8 into DRAM scratch, then transposed
    with the xbar DMA *viewed as uint16* (the xbar only does 2-byte moves):
    partition p of a transposed strip holds the adjacent fp8 column PAIR
    (2p, 2p+1), i.e. contraction index kappa = 256*tau + 2p + t.  A pair of
    strided engine copies de-interleaves each strip into the blocked
    [p, tau, t, n] layout DoubleRow needs.  Both operands of every
    projection matmul use the same kappa mapping, so the contraction is
    order-consistent.
  - projections: qT [e, m], kT [e, c] (bf16 copies out of fp32 PSUM for the
    scores), v natural [c, e] scattered per-head into vA8 (fp8) with an
    appended ones-column (attn@v then also emits the softmax denominator).
  - scores_h^T [c, m] = kT_h^T-tile @ qT_h in bf16 (hd=64 contraction; two
    heads of an e-tile use PE row groups 0-63 / 64-127).
  - P = exp(scale*scores - 2.5) on ACT, fp8 out ([128,1024] tiles); the bias
    keeps P inside fp8e4 range and cancels in the softmax normalization.
  - attn@v: 4 DoubleRow matmuls per head; DVE reciprocal + Pool
    partition-broadcast + DVE mul normalize into attnT8 (fp8, d-blocked).
  - out-projection: Wo goes through the bf16 xbar path and is cast on-chip
    to the d-blocked fp8 layout (matching attnT8's kappa); runs as
    DoubleRow in four d-quarters as heads complete, accumulating into the
    fp32 residual tile (x + bo); finished blocks stream out on ACT HWDGE.
"""

import numpy as np

import concourse.bass as bass
import concourse.tile as tile
from concourse import bacc, mybir
from concourse.bass_utils import run_bass_kernel_spmd

f32 = mybir.dt.float32
bf16 = mybir.dt.bfloat16
fp8 = mybir.dt.float8e4
u16 = mybir.dt.uint16
DR = mybir.MatmulPerfMode.DoubleRow
Exp = mybir.ActivationFunctionType.Exp

B, L, LC, D, CD, H, HD = 2, 2048, 1024, 1024, 768, 16, 64
NCORES = 8
M = (B * L) // NCORES  # 512 query rows per core
MT = M // 128  # 4
DT = D // 128  # 8
CDT = CD // 128  # 6
CT = LC // 128  # 8
ET = D // 128  # 8
TQ = D // 256  # 4  DoubleRow K-passes for d-contractions
TC = CD // 256  # 3  DoubleRow K-passes for cd-contractions
SCALE = float(HD) ** -0.5
VW = 80  # per-head v width in vA8: 64 v cols + ones col at 64 + 15 pad
LAG = 3  # attn@v for head h runs in slot h+LAG (hides exp latency)

LAST_RESULT = None  # BassKernelResults of the most recent run (for test.py)
_cached_nc = None


def _build():
    nc = bacc.Bacc("TRN2", target_bir_lowering=False, debug=False, num_devices=NCORES)
    x_d = nc.dram_tensor("x", [M, D], f32, kind="ExternalInput").ap()
    ctx_d = nc.dram_tensor("ctx", [LC, CD], f32, kind="ExternalInput").ap()
    wq_d = nc.dram_tensor("wq", [D, D], f32, kind="ExternalInput").ap()
    wk_d = nc.dram_tensor("wk", [D, CD], f32, kind="ExternalInput").ap()
    wv_d = nc.dram_tensor("wv", [D, CD], f32, kind="ExternalInput").ap()
    wo_d = nc.dram_tensor("wo", [D, D], f32, kind="ExternalInput").ap()
    bo_d = nc.dram_tensor("bo", [1, D], f32, kind="ExternalInput").ap()
    out_d = nc.dram_tensor("out", [M, D], f32, kind="ExternalOutput").ap()
    out_r = out_d.rearrange("(t p) d -> t p d", p=128)

    with tile.TileContext(nc) as tc:
        with (
            tc.tile_pool(name="const", bufs=1) as const_pool,
            tc.tile_pool(name="persist", bufs=1) as persist,
            tc.tile_pool(name="p8", bufs=LAG + 2) as p8_pool,
            tc.tile_pool(name="r", bufs=4) as r_pool,
            tc.tile_pool(name="dram", bufs=3, space="DRAM") as dram_pool,
            tc.tile_pool(name="mmps", bufs=2, space="PSUM") as mmps,
            tc.tile_pool(name="scps", bufs=2, space="PSUM") as scps,
            tc.tile_pool(name="avps", bufs=2, space="PSUM") as avps,
        ):
            # exp bias keeps P in fp8e4 range (max logit ~6.7 -> e^4.2 = 67
            # < 448); softmax normalization cancels it exactly
            ebias = const_pool.tile([128, 1], f32, tag="ebias")
            nc.gpsimd.memset(ebias[:], -2.5)

            bo_sb = const_pool.tile([1, D], f32, tag="bo")
            nc.scalar.dma_start(bo_sb[:], bo_d)
            bo_b = const_pool.tile([128, D], f32, tag="bo_b")

            # fp8 blocked operands (stationary side): kappa = 256*tau+2p+t;
            # moving operands read the pair-interleaved transpose buffers
            # directly via strided views (same kappa)
            wq8 = persist.tile([128, TQ, 2, D], fp8, tag="wq8")
            wk8 = persist.tile([128, TC, 2, D], fp8, tag="wk8")
            ctx8 = persist.tile([128, TC, 2, LC], fp8, tag="ctx8")
            # bf16 attention operands (from PSUM)
            qT = persist.tile([128, ET, M], bf16, tag="qT")
            kT = persist.tile([128, ET, LC], bf16, tag="kT")
            vA8 = persist.tile([128, H, CT, VW], fp8, tag="vA8")
            attnT8 = persist.tile([128, 4, MT, 2, 128], fp8, tag="attnT8")
            # Wo: bf16 xbar path, then on-chip cast to d-blocked fp8
            woT = persist.tile([128, DT, D], bf16, tag="woT")
            wo8 = persist.tile([128, 4, 2, 2, 512], fp8, tag="wo8")
            acc = persist.tile([128, MT, D], f32, tag="acc")

            # ---- fp8 scratch cast + u16 xbar transpose + de-interleave ----
            scratch = {}
            inter = {}

            def cast_cols(name, src_d, rows, width, lo, hi, after=None):
                if name not in scratch:
                    scratch[name] = dram_pool.tile(
                        [rows, width], fp8, tag="scr", name=f"scr_{name}"
                    )
                c = nc.gpsimd.dma_start(
                    scratch[name][:, lo:hi], src_d[:, lo:hi]
                )
                if after is not None:
                    tile.add_dep_helper(c.ins, after, reason="dma priority")
                return c.ins

            def transpose_deint(name, dest, rows, width, slo, shi, engs=()):
                """u16 transpose of strips [256*slo, 256*shi); if dest is
                given, strided copies de-interleave each strip into
                dest[:,tau,:,:] (engines alternate from engs)."""
                scr_u = scratch[name][:].bitcast(u16)  # [rows, width//2]
                nt = width // 256
                if name not in inter:
                    inter[name] = persist.tile(
                        [128, nt, 2 * rows], fp8, tag=f"inter_{name}", name=f"i8_{name}"
                    )
                it = inter[name]
                tr = nc.sync.dma_start_transpose(
                    out=it[:, slo:shi, :].bitcast(u16),
                    in_=scr_u[:, slo * 128 : shi * 128],
                )
                for i, tau in enumerate(range(slo, shi)):
                    if dest is None:
                        continue
                    iv = it[:, tau, :].rearrange("p (n t) -> p t n", t=2)
                    eng = engs[i % len(engs)]
                    if eng is nc.scalar:
                        nc.scalar.copy(dest[:, tau, :, :], iv[:, :, :])
                    else:
                        eng.tensor_copy(dest[:, tau, :, :], iv[:, :, :])
                return tr.ins

            def iview(name, tau):
                return inter[name][:, tau, :].rearrange("p (n t) -> p t n", t=2)

            # DMA priority chain: q path (wq, x) first so PE starts early;
            # later casts are dep-held so their transfers cannot jump the
            # single DMA device's queue ahead of critical transposes.
            cast_cols("wq", wq_d, D, D, 0, 512)
            cast_cols("x", x_d, M, D, 0, 512)
            t_wq0 = transpose_deint("wq", wq8, D, D, 0, 2, (nc.scalar, nc.vector))
            t_x0 = transpose_deint("x", None, M, D, 0, 2)
            cast_cols("wq", wq_d, D, D, 512, D)
            cast_cols("x", x_d, M, D, 512, D)
            t_wq1 = transpose_deint("wq", wq8, D, D, 2, 4, (nc.scalar, nc.vector))
            t_x1 = transpose_deint("x", None, M, D, 2, 4)
            cast_cols("wk", wk_d, D, CD, 0, CD, after=t_x1)
            cast_cols("ctx", ctx_d, LC, CD, 0, CD, after=t_wq1)
            t_wk = transpose_deint("wk", wk8, D, CD, 0, 3, (nc.vector, nc.scalar))
            t_ctx = transpose_deint("ctx", ctx8, LC, CD, 0, 3, (nc.vector, nc.scalar))
            cast_cols("wv", wv_d, D, CD, 0, CD, after=t_wk)
            t_wv = transpose_deint("wv", None, D, CD, 0, 3)

            # Wo bf16 path (cast -> xbar -> SBUF) and the residual load —
            # needed only mid-attention, held behind the ctx transpose
            wo_scr = dram_pool.tile([D, D], bf16, tag="wo_scr")
            c_wo = nc.gpsimd.dma_start(wo_scr[:], wo_d)
            tile.add_dep_helper(c_wo.ins, t_ctx, reason="dma priority")
            nc.sync.dma_start_transpose(out=woT[:], in_=wo_scr[:])
            c_acc = nc.scalar.dma_start(
                acc[:], x_d.rearrange("(t p) d -> p t d", p=128)
            )
            tile.add_dep_helper(c_acc.ins, t_wv, reason="dma priority")
            nc.gpsimd.partition_broadcast(bo_b[:], bo_sb[:])
            # vA8 pad columns zero, ones column at 64
            nc.gpsimd.memset(vA8[:, :, :, HD:], 0.0)
            nc.gpsimd.memset(vA8[:, :, :, HD : HD + 1], 1.0)
            for mt in range(MT):
                nc.gpsimd.tensor_add(acc[:, mt, :], acc[:, mt, :], bo_b[:])

            # ---- fp8 DoubleRow projection chains ----
            def emit_q(et):
                ps = mmps.tile([128, M], f32, tag="ps", name="qps")
                for tau in range(TQ):
                    nc.tensor.matmul(
                        ps[:],
                        wq8[:, tau, :, et * 128 : (et + 1) * 128],
                        iview("x", tau),
                        start=(tau == 0),
                        stop=(tau == TQ - 1),
                        perf_mode=DR,
                    )
                if et < 3:
                    nc.scalar.copy(qT[:, et, :], ps[:])
                else:
                    nc.vector.tensor_copy(qT[:, et, :], ps[:])

            def emit_k(et, cc):
                ps = mmps.tile([128, 512], f32, tag="ps", name="kps")
                for tau in range(TC):
                    nc.tensor.matmul(
                        ps[:],
                        wk8[:, tau, :, et * 128 : (et + 1) * 128],
                        iview("ctx", tau)[:, :, cc * 512 : (cc + 1) * 512],
                        start=(tau == 0),
                        stop=(tau == TC - 1),
                        perf_mode=DR,
                    )
                if et < 1:
                    nc.scalar.copy(kT[:, et, cc * 512 : (cc + 1) * 512], ps[:])
                else:
                    nc.vector.tensor_copy(kT[:, et, cc * 512 : (cc + 1) * 512], ps[:])

            def emit_v(ct, ec):
                ps = mmps.tile([128, 512], f32, tag="ps", name="vps")
                for tau in range(TC):
                    nc.tensor.matmul(
                        ps[:],
                        ctx8[:, tau, :, ct * 128 : (ct + 1) * 128],
                        iview("wv", tau)[:, :, ec * 512 : (ec + 1) * 512],
                        start=(tau == 0),
                        stop=(tau == TC - 1),
                        perf_mode=DR,
                    )
                nc.vector.tensor_copy(
                    vA8[:, ec * 8 : (ec + 1) * 8, ct, 0:HD],
                    ps[:].rearrange("p (h w) -> p h w", w=HD),
                )

            def emit_wo8(tau, ec):
                nc.gpsimd.tensor_copy(
                    wo8[:, tau, ec, :, :],
                    woT[:, 2 * tau : 2 * tau + 2, ec * 512 : (ec + 1) * 512],
                )

            # ---- attention emitters ----
            p8_tiles = {}

            def emit_scores_exp(h):
                et, half = h // 2, h % 2
                rows = slice(half * HD, (half + 1) * HD)
                p8 = p8_pool.tile([128, TQ, LC], fp8, tag="p8", name="p8t")
                p8_tiles[h] = p8
                for ctp in range(4):
                    sc = scps.tile([128, 1024], f32, tag="sc", name="scs")
                    for k2 in range(2):
                        ct = 2 * ctp + k2
                        nc.tensor.matmul(
                            sc[:, k2 * 512 : (k2 + 1) * 512],
                            kT[rows, et, ct * 128 : (ct + 1) * 128],
                            qT[rows, et, :],
                            start=True,
                            stop=True,
                        )
                    nc.scalar.activation(
                        out=p8[:, ctp, :], in_=sc[:], func=Exp, scale=SCALE,
                        bias=ebias[:],
                    )

            def emit_av_norm(h):
                rows = slice((h % 2) * HD, (h % 2 + 1) * HD)
                p8 = p8_tiles.pop(h)
                av = avps.tile([VW, 512], f32, tag="av", name="avt")
                for tau in range(4):
                    nc.tensor.matmul(
                        av[:],
                        vA8[:, h, 2 * tau : 2 * tau + 2, :],
                        p8[:, tau, :].rearrange("p (t m) -> p t m", t=2),
                        start=(tau == 0),
                        stop=(tau == 3),
                        perf_mode=DR,
                    )
                rcp = r_pool.tile([1, 512], f32, tag="r")
                nc.vector.reciprocal(rcp[:], av[HD : HD + 1, :])
                rcp_b = r_pool.tile([HD, 512], f32, tag="rb")
                nc.gpsimd.partition_broadcast(rcp_b[:], rcp[:])
                tau, t = h // 4, (h % 4) // 2
                nc.vector.tensor_mul(
                    attnT8[rows, tau, :, t, :],
                    av[0:HD, :].rearrange("p (mt j) -> p mt j", j=128),
                    rcp_b[:].rearrange("p (mt j) -> p mt j", j=128),
                )

            def out_piece(tau, piece):
                mt, ec = piece // 2, piece % 2
                ps = mmps.tile([128, 512], f32, tag="ps", name="ops")
                nc.tensor.matmul(
                    ps[:],
                    attnT8[:, tau, mt, :, :],
                    wo8[:, tau, ec, :, :],
                    start=True,
                    stop=True,
                    perf_mode=DR,
                )
                sl = slice(ec * 512, (ec + 1) * 512)
                nc.vector.tensor_add(acc[:, mt, sl], acc[:, mt, sl], ps[:])
                if tau == 3:
                    nc.sync.dma_start(out_r[mt][:, sl], acc[:, mt, sl])

            # ---- PE program ----
            # preamble: q chains (strip deps let tau-passes start as x/wq
            # strips land), k et0-1, v ec0 ct0-3
            for et in range(ET):
                emit_q(et)
            for et in range(2):
                emit_k(et, 0)
                emit_k(et, 1)
            for ct in range(4):
                emit_v(ct, 0)

            chains = {
                0: [lambda: emit_v(4, 0), lambda: emit_v(5, 0)],
                1: [lambda: emit_v(6, 0), lambda: emit_v(7, 0)],
                2: [lambda: emit_k(2, 0), lambda: emit_k(2, 1)],
                3: [lambda: emit_k(3, 0), lambda: emit_k(3, 1)],
                4: [lambda: emit_v(0, 1), lambda: emit_v(1, 1)],
                5: [lambda: emit_v(2, 1), lambda: emit_v(3, 1),
                    lambda: emit_wo8(0, 0), lambda: emit_wo8(0, 1)],
                6: [lambda: emit_k(4, 0), lambda: emit_k(4, 1),
                    lambda: emit_wo8(1, 0), lambda: emit_wo8(1, 1)],
                7: [lambda: emit_v(4, 1), lambda: emit_v(5, 1),
                    lambda: emit_wo8(2, 0), lambda: emit_wo8(2, 1)],
                8: [lambda: emit_v(6, 1), lambda: emit_v(7, 1),
                    lambda: emit_wo8(3, 0), lambda: emit_wo8(3, 1)],
                9: [lambda: emit_k(5, 0), lambda: emit_k(5, 1)],
                10: [lambda: emit_k(6, 0), lambda: emit_k(6, 1)],
                11: [lambda: emit_k(7, 0), lambda: emit_k(7, 1)],
            }
            pieces = {
                9: [(0, 0), (0, 1), (0, 2), (0, 3)],
                10: [(0, 4), (0, 5), (0, 6), (0, 7)],
                11: [(1, 0), (1, 1), (1, 2), (1, 3)],
                12: [(1, 4), (1, 5), (1, 6), (1, 7)],
                15: [(2, 0), (2, 1), (2, 2), (2, 3)],
                16: [(2, 4), (2, 5), (2, 6), (2, 7)],
                18: [(3, 0), (3, 1), (3, 2), (3, 3)],
                19: [(3, 4), (3, 5), (3, 6), (3, 7)],
            }

            for slot in range(H + LAG + 2):
                if slot < H:
                    emit_scores_exp(slot)
                for fn in chains.get(slot, ()):
                    fn()
                if slot >= LAG and slot - LAG < H:
                    emit_av_norm(slot - LAG)
                for tau, piece in pieces.get(slot, ()):
                    out_piece(tau, piece)

    nc.compile()
    return nc


def kernel(x, context, Wq, Wk, Wv, Wo, bo):
    global LAST_RESULT, _cached_nc
    if _cached_nc is None:
        _cached_nc = _build()
    nc = _cached_nc

    x = np.ascontiguousarray(x, dtype=np.float32)
    context = np.ascontiguousarray(context, dtype=np.float32)
    wq = np.ascontiguousarray(Wq, dtype=np.float32)
    wk = np.ascontiguousarray(Wk, dtype=np.float32)
    wv = np.ascontiguousarray(Wv, dtype=np.float32)
    wo = np.ascontiguousarray(Wo, dtype=np.float32)
    bo2 = np.ascontiguousarray(bo, dtype=np.float32).reshape(1, D)

    in_maps = []
    for c in range(NCORES):
        b = c // (NCORES // B)
        ls = (c % (NCORES // B)) * M
        in_maps.append(
            {
                "x": np.ascontiguousarray(x[b, ls : ls + M, :]),
                "ctx": context[b],
                "wq": wq,
                "wk": wk,
                "wv": wv,
                "wo": wo,
                "bo": bo2,
            }
        )

    res = run_bass_kernel_spmd(nc, in_maps, core_ids=list(range(NCORES)))
    LAST_RESULT = res

    out = np.empty((B, L, D), dtype=np.float32)
    for c in range(NCORES):
        b = c // (NCORES // B)
        ls = (c % (NCORES // B)) * M
        out[b, ls : ls + M, :] = res.results[c]["out"]
    return out
